# revision 1
# baseline (speedup 1.0000x reference)
"""Trainium2 Bass kernel for nn_LogicLayer (soft logic-gate mixture layer).

Reference computation:
    p = softmax(weights, axis=-1)            # [OUT, 16]
    c = p @ GATE_COEF                        # [OUT, 4]
    a = x[:, idx0]; b = x[:, idx1]           # [B, OUT]
    out = c0 + c1*a + c2*b + c3*a*b

Strategy (data-parallel over batch, 8 cores, 512 rows each):
  Host: fold softmax+coef into c[OUT,4]; build int16 wrapped index tables.
  Device, per core:
    Phase 1: stream x shard [512, 8192] in feature chunks, PE-transpose to
             xT [8192, 512] in DRAM.
    Phase 2: dma_gather rows of xT for idx0/idx1 (2 KiB/row descriptors),
             compute out^T = (c1*a + c0) + (c3*a + c2)*b with per-partition
             scalars on ACT + elementwise on DVE, store out^T [8192, 512].
  Host: transpose + concat per-core out^T slices into out [4096, 8192].
"""

import numpy as np

B, IN_DIM, OUT_DIM = 4096, 8192, 8192
N_CORES = 8
BSH = B // N_CORES  # 512 batch rows per core

GATE_COEF = np.array([
    [0.,  0.,  0.,  0.],
    [0.,  0.,  0.,  1.],
    [0.,  1.,  0., -1.],
    [0.,  1.,  0.,  0.],
    [0.,  0.,  1., -1.],
    [0.,  0.,  1.,  0.],
    [0.,  1.,  1., -2.],
    [0.,  1.,  1., -1.],
    [1., -1., -1.,  1.],
    [1., -1., -1.,  2.],
    [1.,  0., -1.,  0.],
    [1.,  0., -1.,  1.],
    [1., -1.,  0.,  0.],
    [1., -1.,  0.,  1.],
    [1.,  0.,  0., -1.],
    [1.,  0.,  0.,  0.],
], dtype=np.float32)

_NC_CACHE = {}


def build_nc(bsh=BSH, in_dim=IN_DIM, out_dim=OUT_DIM, jgroup=1024, fchunk=2048,
             loop_n=1, timing=False, p1_reps=1, p2_reps=1, overlap=False):
    """Build the per-core Bass program (SPMD: same program on all cores).

    loop_n > 1 repeats the whole body in a device-side For_i loop (for
    timing); timing=True keeps only tiny tensors as external I/O so the
    per-call transfer cost is constant.
    """
    import concourse.bacc as bacc
    import concourse.mybir as mybir
    import concourse.tile as tile
    from concourse.masks import make_identity

    f32 = mybir.dt.float32
    i16 = mybir.dt.int16
    AF = mybir.ActivationFunctionType
    OP = mybir.AluOpType

    nbt = bsh // 128        # batch tiles (partition tiles of x)
    fchunk = min(fchunk, in_dim)
    nfc = in_dim // fchunk  # feature chunks for phase-1 streaming
    nfb_c = fchunk // 128   # feature blocks per chunk
    njb = out_dim // 128    # output-column blocks
    jgroup = min(jgroup, out_dim)
    ngr = out_dim // jgroup  # gather groups
    spg = jgroup // 128      # 128-col slots per group

    nc = bacc.Bacc("TRN2", target_bir_lowering=False, debug=False)
    big = "Internal" if timing else None
    x = nc.dram_tensor("x", [bsh, in_dim], f32,
                       kind=big or "ExternalInput")
    ctab = nc.dram_tensor("ctab", [128, njb * 4], f32, kind="ExternalInput")
    idx0w = nc.dram_tensor("idx0w", [128, out_dim // 16], i16, kind="ExternalInput")
    idx1w = nc.dram_tensor("idx1w", [128, out_dim // 16], i16, kind="ExternalInput")
    if overlap:
        xTh = [nc.dram_tensor(f"xT{h}", [in_dim, bsh // 2], f32, kind="Internal")
               for h in range(2)]
    else:
        xT = nc.dram_tensor("xT", [in_dim, bsh], f32, kind="Internal")
    outT = nc.dram_tensor("outT", [out_dim, bsh], f32,
                          kind=big or "ExternalOutput")
    dummy = None
    if timing:
        dummy = nc.dram_tensor("tout", [128, 128], f32, kind="ExternalOutput")

    with tile.TileContext(nc) as tc:
        with (
            tc.tile_pool(name="const", bufs=1) as cpool,
            tc.tile_pool(name="xin", bufs=2) as xpool,
            tc.tile_pool(name="xtout", bufs=4) as xtpool,
            tc.tile_pool(name="psum", bufs=4, space="PSUM") as pspool,
            tc.tile_pool(name="gather", bufs=2) as gpool,
            tc.tile_pool(name="tmp", bufs=4) as tpool,
        ):
            ident = cpool.tile([128, 128], f32)
            make_identity(nc, ident)
            ctab_sb = cpool.tile([128, njb * 4], f32)
            nc.sync.dma_start(ctab_sb, ctab[:, :])
            idx0_sb = cpool.tile([128, out_dim // 16], i16)
            nc.sync.dma_start(idx0_sb, idx0w[:, :])
            idx1_sb = cpool.tile([128, out_dim // 16], i16)
            nc.sync.dma_start(idx1_sb, idx1w[:, :])

            def body():
                if overlap:
                    for _p1 in range(p1_reps):
                        phase1_half(0)
                        phase1_half(1)
                    for _p2 in range(p2_reps):
                        phase2_overlap()
                    return
                # ---- Phase 1: transpose x -> xT (DRAM) ----
                for _p1 in range(p1_reps):
                    phase1()
                # ---- Phase 2: gather + elementwise ----
                for _p2 in range(p2_reps):
                    phase2()

            def phase1_half(h):
                # transpose batch half h -> xTh[h] (all features)
                hb = bsh // 2
                for fc in range(nfc):
                    xin = xpool.tile([128, nbt // 2, fchunk], f32, tag="xin")
                    for bt in range(nbt // 2):
                        nc.sync.dma_start(
                            xin[:, bt, :],
                            x[h * hb + bt * 128:h * hb + (bt + 1) * 128,
                              fc * fchunk:(fc + 1) * fchunk],
                        )
                    for fbl in range(nfb_c):
                        fb = fc * nfb_c + fbl
                        ps = pspool.tile([128, hb], f32, tag="ps")
                        for bt in range(nbt // 2):
                            nc.tensor.transpose(
                                ps[:, bt * 128:(bt + 1) * 128],
                                xin[:, bt, fbl * 128:(fbl + 1) * 128],
                                ident,
                            )
                        xt_sb = xtpool.tile([128, hb], f32, tag="xt")
                        nc.scalar.copy(xt_sb, ps)
                        nc.sync.dma_start(
                            xTh[h][fb * 128:(fb + 1) * 128, :], xt_sb)

            def phase2_overlap():
                hb = bsh // 2
                icols = jgroup // 16
                for g in range(ngr):
                    a_sb = gpool.tile([128, 2, spg, hb], f32, tag="ga")
                    b_sb = gpool.tile([128, 2, spg, hb], f32, tag="gb")
                    for h in range(2):
                        nc.gpsimd.dma_gather(
                            a_sb[:, h], xTh[h][:, :],
                            idx0_sb[:, g * icols:(g + 1) * icols],
                            jgroup, jgroup, hb,
                        )
                        nc.gpsimd.dma_gather(
                            b_sb[:, h], xTh[h][:, :],
                            idx1_sb[:, g * icols:(g + 1) * icols],
                            jgroup, jgroup, hb,
                        )
                    o_sb = gpool.tile([128, 2, spg, hb], f32, tag="go")
                    for s in range(spg):
                        jb = g * spg + s
                        u = tpool.tile([128, 2, hb], f32, tag="u")
                        v = tpool.tile([128, 2, hb], f32, tag="v")
                        nc.scalar.activation(
                            u, a_sb[:, :, s, :], AF.Identity,
                            bias=ctab_sb[:, jb * 4 + 0:jb * 4 + 1],
                            scale=ctab_sb[:, jb * 4 + 1:jb * 4 + 2],
                        )
                        nc.scalar.activation(
                            v, a_sb[:, :, s, :], AF.Identity,
                            bias=ctab_sb[:, jb * 4 + 2:jb * 4 + 3],
                            scale=ctab_sb[:, jb * 4 + 3:jb * 4 + 4],
                        )
                        nc.vector.tensor_tensor(v, v, b_sb[:, :, s, :], OP.mult)
                        nc.vector.tensor_tensor(o_sb[:, :, s, :], v, u, OP.add)
                    og = outT[g * jgroup:(g + 1) * jgroup, :].rearrange(
                        "(s p) (h c) -> p h s c", p=128, h=2
                    )
                    nc.sync.dma_start(og, o_sb[:, :, :, :])

            def phase1():
                for fc in range(nfc):
                    xin = xpool.tile([128, nbt, fchunk], f32, tag="xin")
                    for bt in range(nbt):
                        nc.sync.dma_start(
                            xin[:, bt, :],
                            x[bt * 128:(bt + 1) * 128,
                              fc * fchunk:(fc + 1) * fchunk],
                        )
                    for fbl in range(nfb_c):
                        fb = fc * nfb_c + fbl
                        ps = pspool.tile([128, nbt * 128], f32, tag="ps")
                        for bt in range(nbt):
                            nc.tensor.transpose(
                                ps[:, bt * 128:(bt + 1) * 128],
                                xin[:, bt, fbl * 128:(fbl + 1) * 128],
                                ident,
                            )
                        xt_sb = xtpool.tile([128, bsh], f32, tag="xt")
                        nc.scalar.copy(xt_sb, ps)
                        nc.sync.dma_start(xT[fb * 128:(fb + 1) * 128, :], xt_sb)

            def phase2():
                icols = jgroup // 16  # idx-table columns per group
                for g in range(ngr):
                    a_sb = gpool.tile([128, spg, bsh], f32, tag="ga")
                    b_sb = gpool.tile([128, spg, bsh], f32, tag="gb")
                    nc.gpsimd.dma_gather(
                        a_sb[:, :, :], xT[:, :],
                        idx0_sb[:, g * icols:(g + 1) * icols],
                        jgroup, jgroup, bsh,
                    )
                    nc.gpsimd.dma_gather(
                        b_sb[:, :, :], xT[:, :],
                        idx1_sb[:, g * icols:(g + 1) * icols],
                        jgroup, jgroup, bsh,
                    )
                    o_sb = gpool.tile([128, spg, bsh], f32, tag="go")
                    for s in range(spg):
                        jb = g * spg + s
                        u = tpool.tile([128, bsh], f32, tag="u")
                        v = tpool.tile([128, bsh], f32, tag="v")
                        # u = c1*a + c0 ; v = c3*a + c2 (per-partition scalars)
                        nc.scalar.activation(
                            u, a_sb[:, s], AF.Identity,
                            bias=ctab_sb[:, jb * 4 + 0:jb * 4 + 1],
                            scale=ctab_sb[:, jb * 4 + 1:jb * 4 + 2],
                        )
                        nc.scalar.activation(
                            v, a_sb[:, s], AF.Identity,
                            bias=ctab_sb[:, jb * 4 + 2:jb * 4 + 3],
                            scale=ctab_sb[:, jb * 4 + 3:jb * 4 + 4],
                        )
                        nc.vector.tensor_tensor(v, v, b_sb[:, s], OP.mult)
                        nc.vector.tensor_tensor(o_sb[:, s], v, u, OP.add)
                    og = outT[g * jgroup:(g + 1) * jgroup, :].rearrange(
                        "(s p) c -> p s c", p=128
                    )
                    nc.sync.dma_start(og, o_sb[:, :, :])

            if loop_n > 1:
                with tc.For_i(0, loop_n) as _i:
                    body()
            else:
                body()

            if dummy is not None:
                nc.sync.dma_start(dummy[:, :], ctab_sb[:, 0:128])

    nc.compile()
    return nc


def host_prep(weights, idx0, idx1, out_dim=OUT_DIM):
    """Fold softmax+gate coefficients; build wrapped int16 index tables."""
    w = np.asarray(weights, dtype=np.float32)
    m = w.max(axis=-1, keepdims=True)
    e = np.exp(w - m, dtype=np.float32)
    p = e / e.sum(axis=-1, keepdims=True, dtype=np.float32)
    c = (p @ GATE_COEF).astype(np.float32)  # [out_dim, 4]
    njb = out_dim // 128
    # ctab[p, jb*4+k] = c[jb*128+p, k]
    ctab = np.ascontiguousarray(
        c.reshape(njb, 128, 4).transpose(1, 0, 2).reshape(128, njb * 4)
    )

    def wrap(idx):
        idx = np.asarray(idx).astype(np.int16)
        t = idx.reshape(out_dim // 16, 16).T  # [16, cols]; t[p, col] = idx[col*16+p]
        return np.ascontiguousarray(np.tile(t, (8, 1)))  # replicate to 128 partitions

    return ctab, wrap(idx0), wrap(idx1)


def kernel(x, weights, idx0, idx1):
    from concourse.bass_utils import run_bass_kernel_spmd

    x = np.ascontiguousarray(np.asarray(x, dtype=np.float32))
    ctab, i0w, i1w = host_prep(weights, idx0, idx1)

    if "nc" not in _NC_CACHE:
        _NC_CACHE["nc"] = build_nc()
    nc = _NC_CACHE["nc"]

    in_maps = [
        {
            "x": x[c * BSH:(c + 1) * BSH],
            "ctab": ctab,
            "idx0w": i0w,
            "idx1w": i1w,
        }
        for c in range(N_CORES)
    ]
    res = run_bass_kernel_spmd(nc, in_maps, core_ids=list(range(N_CORES)))
    out = np.empty((B, OUT_DIM), dtype=np.float32)
    for c in range(N_CORES):
        out[c * BSH:(c + 1) * BSH] = res.results[c]["outT"].T
    return out

